# revision 2
# baseline (speedup 1.0000x reference)
"""AddShift_mp_linear_module on 8 TRN2 NeuronCores.

Strategy (channel-block sharding, no collectives):
  - 96 output-channel blocks (11 input channels each) -> 12 blocks/core.
  - Every branch is a contraction over the block's (k, spatial) axis:
      out_v[co, h, (b,w)]  = sum_{k,h'} Ov[(k,h'), h]   * x[b, c, h', w]
      out_i[co, h, (b,w)]  = sum_{k,h'} Oi[(k,h'), h]   * x[b, c, h', w]
      out_h[co, w, (b,h')] = sum_{k,w'} Oh[(k,w'), w]   * x[b, c, h', w']
    where the sparse operators Ov/Oi/Oh are built on the host from
    w1/w2/w3/pad_hv/idx_identit (all known at call time).
  - On device: per block, 6 PSUM-accumulated matmuls over K-chunks of the
    660-row contraction (5x128 + 20). V and I share one matmul chain
    (stationary [K,112] = [V | I] columns). H uses a host-pretransposed
    copy of x (w-major) since the tensor engine contracts partitions.
  - x is cast to bf16 on host; batch rides in the matmul free dim (8*60=480).
  - Outputs return as [56, 480] fp32 tiles; host restores [b, co, h, w].
"""

import numpy as np
import ml_dtypes

# architecture constants (match reference init_kwargs)
B = 8
C_OUT = 96
NK = 11
G = 4
C_IN = C_OUT * NK          # 1056
HOUT = WOUT = 56
HIN = WIN = 60
EP = 2                     # extra pad
N_CORES = 8
BPC = C_OUT // N_CORES     # blocks per core = 12
CPC = BPC * NK             # channels per core = 132
KROWS = NK * HIN           # 660 contraction rows per block
KCH = 128                  # K-chunk size
NCHUNK = (KROWS + KCH - 1) // KCH   # 6 (5x128 + 20)
NFREE = B * WIN            # 480 matmul free dim

BF16 = ml_dtypes.bfloat16

_CACHE = {}


def _build_operators(w1, w2, w3, pad_hv, idx_identit):
    """Build per-block stationary operators.

    Returns opv (96, 660, 112) fp32  [cols 0:56 = V, 56:112 = identity]
            oph (96, 660, 56)  fp32
    Row r = k*60 + spatial_in, for channel c = co*11 + k.
    """
    w1r = np.asarray(w1, np.float32).reshape(G, C_IN)
    w2r = np.asarray(w2, np.float32).reshape(G, C_IN)
    w3r = np.asarray(w3, np.float32).reshape(G, C_OUT)
    pad = np.asarray(pad_hv, np.int64)            # (C_IN, 2G)
    idx = np.asarray(idx_identit, np.int64)       # (C_OUT, G)

    opv = np.zeros((C_OUT, KROWS, 112), np.float32)
    oph = np.zeros((C_OUT, KROWS, 56), np.float32)

    c_all = np.arange(C_IN)
    co_all = c_all // NK
    k_all = c_all % NK
    pos = np.arange(HOUT)                          # output spatial index

    for g in range(G):
        # horizontal: w_in = w_out + EP + pad[c, g]
        win = pos[None, :] + EP + pad[:, g][:, None]        # (C_IN, 56)
        ok = (win >= 0) & (win < WIN)
        cc, oo = np.nonzero(ok)
        np.add.at(oph, (co_all[cc], k_all[cc] * HIN + win[cc, oo], oo), w1r[g, cc])
        # vertical: h_in = h_out + EP + pad[c, G+g]
        hin = pos[None, :] + EP + pad[:, G + g][:, None]
        ok = (hin >= 0) & (hin < HIN)
        cc, oo = np.nonzero(ok)
        np.add.at(opv, (co_all[cc], k_all[cc] * HIN + hin[cc, oo], oo), w2r[g, cc])

    # identity: out_i[co] = sum_g w3r[g, co] * x[idx[co, g]] (idx within block co)
    k_sel = idx - np.arange(C_OUT)[:, None] * NK            # (C_OUT, G)
    assert np.all((k_sel >= 0) & (k_sel < NK)), "idx_identit outside its block"
    u = np.zeros((C_OUT, NK), np.float32)
    for g in range(G):
        np.add.at(u, (np.arange(C_OUT), k_sel[:, g]), w3r[g])
    co_i, k_i = np.nonzero(u != 0)
    for co, k in zip(co_i, k_i):
        opv[co, k * HIN + pos + EP, 56 + pos] += u[co, k]
    return opv, oph


def _chunk_ops(op):
    """(96, 660, M) -> per-core (BPC, 128, NCHUNK, M) bf16, zero-padded."""
    m = op.shape[2]
    padded = np.zeros((C_OUT, NCHUNK * KCH, m), np.float32)
    padded[:, :KROWS] = op
    # (96, NCHUNK, 128, m) -> (96, 128, NCHUNK, m)
    r = padded.reshape(C_OUT, NCHUNK, KCH, m).transpose(0, 2, 1, 3)
    return np.ascontiguousarray(r.astype(BF16))


def _build_nc():
    import concourse.bacc as bacc
    import concourse.tile as tile
    import concourse.bass as bass
    import concourse.mybir as mybir
    from contextlib import ExitStack

    f32 = mybir.dt.float32
    bf16 = mybir.dt.bfloat16

    nc = bacc.Bacc(None, target_bir_lowering=False)
    xv_d = nc.declare_dram_parameter("xv", [CPC * HIN, NFREE], bf16, isOutput=False)
    xh_d = nc.declare_dram_parameter("xh", [CPC * WIN, NFREE], bf16, isOutput=False)
    opv_d = nc.declare_dram_parameter("opv", [BPC, KCH, NCHUNK, 112], bf16, isOutput=False)
    oph_d = nc.declare_dram_parameter("oph", [BPC, KCH, NCHUNK, 56], bf16, isOutput=False)
    out_d = nc.declare_dram_parameter("out", [3, BPC, 56, NFREE], f32, isOutput=True)

    with tile.TileContext(nc) as tc, ExitStack() as ctx:
        rhs_pool = ctx.enter_context(tc.tile_pool(name="rhs", bufs=8))
        op_pool = ctx.enter_context(tc.tile_pool(name="ops", bufs=3))
        o_pool = ctx.enter_context(tc.tile_pool(name="outs", bufs=3))
        psum_pool = ctx.enter_context(
            tc.tile_pool(name="psum", bufs=2, space=bass.MemorySpace.PSUM)
        )
        for bi in range(BPC):
            opv_t = op_pool.tile([KCH, NCHUNK, 112], bf16, tag="opv")
            nc.sync.dma_start(opv_t[:], opv_d[bi])
            psum_vi = psum_pool.tile([112, NFREE], f32, tag="pv")
            for j in range(NCHUNK):
                k = min(KCH, KROWS - j * KCH)
                r0 = bi * KROWS + j * KCH
                rv = rhs_pool.tile([KCH, NFREE], bf16, tag="rv")
                nc.sync.dma_start(rv[:k], xv_d[r0:r0 + k, :])
                nc.tensor.matmul(
                    psum_vi[:], opv_t[:k, j, :], rv[:k, :],
                    start=(j == 0), stop=(j == NCHUNK - 1),
                )
            ov = o_pool.tile([112, NFREE], f32, tag="ov")
            nc.vector.tensor_copy(ov[:], psum_vi[:])
            nc.sync.dma_start(out_d[0, bi], ov[:56])
            nc.sync.dma_start(out_d[1, bi], ov[56:112])

            oph_t = op_pool.tile([KCH, NCHUNK, 56], bf16, tag="oph")
            nc.sync.dma_start(oph_t[:], oph_d[bi])
            psum_h = psum_pool.tile([56, NFREE], f32, tag="ph")
            for j in range(NCHUNK):
                k = min(KCH, KROWS - j * KCH)
                r0 = bi * KROWS + j * KCH
                rh = rhs_pool.tile([KCH, NFREE], bf16, tag="rh")
                nc.sync.dma_start(rh[:k], xh_d[r0:r0 + k, :])
                nc.tensor.matmul(
                    psum_h[:], oph_t[:k, j, :], rh[:k, :],
                    start=(j == 0), stop=(j == NCHUNK - 1),
                )
            oh = o_pool.tile([56, NFREE], f32, tag="oh")
            nc.scalar.copy(oh[:], psum_h[:])
            nc.sync.dma_start(out_d[2, bi], oh[:])
    nc.finalize()
    return nc


def prepare_inputs(x, w1, w2, w3, pad_hv, idx_identit):
    """Host-side shard prep. Returns in_maps (list of 8 dicts)."""
    x = np.asarray(x)
    xb = x.astype(BF16)                                   # (B, C, 60, 60)
    # h-major for V/I: [c, h, (b, w)]
    x_hbw = np.ascontiguousarray(xb.transpose(1, 2, 0, 3)).reshape(C_IN * HIN, NFREE)
    # w-major for H: [c, w, (b, h)]
    x_wbh = np.ascontiguousarray(xb.transpose(1, 3, 0, 2)).reshape(C_IN * WIN, NFREE)

    opv, oph = _build_operators(w1, w2, w3, pad_hv, idx_identit)
    opv_c = _chunk_ops(opv)                                # (96, 128, 6, 112)
    oph_c = _chunk_ops(oph)                                # (96, 128, 6, 56)

    in_maps = []
    for i in range(N_CORES):
        r0 = i * CPC * HIN
        in_maps.append({
            "xv": x_hbw[r0:r0 + CPC * HIN],
            "xh": x_wbh[r0:r0 + CPC * WIN],
            "opv": np.ascontiguousarray(opv_c[i * BPC:(i + 1) * BPC]),
            "oph": np.ascontiguousarray(oph_c[i * BPC:(i + 1) * BPC]),
        })
    return in_maps


def unshard(results):
    """results: list of 8 dicts with 'out' (3, BPC, 56, 480) fp32 ->
    (out_h, out_v, out_i) each (B, C_OUT, 56, 56) fp32."""
    O = np.stack([np.asarray(r["out"], np.float32) for r in results])  # (8,3,12,56,480)
    O = O.reshape(N_CORES, 3, BPC, 56, B, 60)
    vi = O[:, 0:2, :, :, :, EP:EP + WOUT]     # (core, 2, co_l, h, b, w)
    # (core, co_l, h, b, w) -> (b, core, co_l, h, w)
    out_v = vi[:, 0].transpose(3, 0, 1, 2, 4).reshape(B, C_OUT, HOUT, WOUT)
    out_i = vi[:, 1].transpose(3, 0, 1, 2, 4).reshape(B, C_OUT, HOUT, WOUT)
    h = O[:, 2, :, :, :, EP:EP + HOUT]        # (core, co_l, w, b, h)
    out_h = h.transpose(3, 0, 1, 4, 2).reshape(B, C_OUT, HOUT, WOUT)
    return out_h, out_v, out_i


def kernel(x, w1, w2, w3, pad_hv, idx_identit, b=B, hout=HOUT, wout=WOUT):
    from concourse.bass_utils import run_bass_kernel_spmd

    assert int(b) == B and int(hout) == HOUT and int(wout) == WOUT
    assert tuple(np.asarray(x).shape) == (B, C_IN, HIN, WIN)

    in_maps = prepare_inputs(x, w1, w2, w3, pad_hv, idx_identit)
    nc = _CACHE.get("nc")
    if nc is None:
        nc = _build_nc()
        _CACHE["nc"] = nc
    res = run_bass_kernel_spmd(nc, in_maps, core_ids=list(range(N_CORES)))
    return unshard(res.results)


# revision 8
# speedup vs baseline: 1.2218x; 1.2218x over previous
"""AddShift_mp_linear_module on 8 TRN2 NeuronCores.

Strategy (channel-block sharding, no collectives):
  - 96 output-channel blocks (11 input channels each) -> 12 blocks/core.
  - Every branch is a contraction over the block's (k, spatial) axis:
      out_v[co, h, (b,w)]  = sum_{k,h'} Ov[(k,h'), h]   * x[b, c, h', w]
      out_i[co, h, (b,w)]  = sum_{k,h'} Oi[(k,h'), h]   * x[b, c, h', w]
      out_h[co, w, (b,h')] = sum_{k,w'} Oh[(k,w'), w]   * x[b, c, h', w']
    where the sparse operators Ov/Oi/Oh are built on the host from
    w1/w2/w3/pad_hv/idx_identit (all known at call time).
  - On device: per block, 6 PSUM-accumulated matmuls over K-chunks of the
    660-row contraction (5x128 + 20). V and I share one matmul chain
    (stationary [K,112] = [V | I] columns). H uses a host-pretransposed
    copy of x (w-major) since the tensor engine contracts partitions.
  - x is cast to bf16 on host; batch rides in the matmul free dim (8*60=480).
  - Outputs return as [56, 480] fp32 tiles; host restores [b, co, h, w].
"""

import numpy as np
import ml_dtypes

# architecture constants (match reference init_kwargs)
B = 8
C_OUT = 96
NK = 11
G = 4
C_IN = C_OUT * NK          # 1056
HOUT = WOUT = 56
HIN = WIN = 60
EP = 2                     # extra pad
N_CORES = 8
BPC = C_OUT // N_CORES     # blocks per core = 12
CPC = BPC * NK             # channels per core = 132
KROWS = NK * HIN           # 660 contraction rows per block
KCH = 110                  # K-chunk size (660 = 6*110, uniform)
NCHUNK = KROWS // KCH      # 6
NFREE = B * WIN            # 480 matmul free dim

BF16 = ml_dtypes.bfloat16

_CACHE = {}


def _build_operators(w1, w2, w3, pad_hv, idx_identit):
    """Build per-block stationary operators.

    Returns opv (96, 660, 120) fp32  [cols 0:56 = V, 64:120 = identity]
            oph (96, 660, 56)  fp32
    Row r = k*60 + spatial_in, for channel c = co*11 + k.
    """
    w1r = np.asarray(w1, np.float32).reshape(G, C_IN)
    w2r = np.asarray(w2, np.float32).reshape(G, C_IN)
    w3r = np.asarray(w3, np.float32).reshape(G, C_OUT)
    pad = np.asarray(pad_hv, np.int64)            # (C_IN, 2G)
    idx = np.asarray(idx_identit, np.int64)       # (C_OUT, G)

    opv = np.zeros((C_OUT, KROWS, 120), np.float32)
    oph = np.zeros((C_OUT, KROWS, 56), np.float32)

    c_all = np.arange(C_IN)
    co_all = c_all // NK
    k_all = c_all % NK
    pos = np.arange(HOUT)                          # output spatial index

    for g in range(G):
        # horizontal: w_in = w_out + EP + pad[c, g]
        win = pos[None, :] + EP + pad[:, g][:, None]        # (C_IN, 56)
        ok = (win >= 0) & (win < WIN)
        cc, oo = np.nonzero(ok)
        np.add.at(oph, (co_all[cc], k_all[cc] * HIN + win[cc, oo], oo), w1r[g, cc])
        # vertical: h_in = h_out + EP + pad[c, G+g]
        hin = pos[None, :] + EP + pad[:, G + g][:, None]
        ok = (hin >= 0) & (hin < HIN)
        cc, oo = np.nonzero(ok)
        np.add.at(opv, (co_all[cc], k_all[cc] * HIN + hin[cc, oo], oo), w2r[g, cc])

    # identity: out_i[co] = sum_g w3r[g, co] * x[idx[co, g]] (idx within block co)
    k_sel = idx - np.arange(C_OUT)[:, None] * NK            # (C_OUT, G)
    assert np.all((k_sel >= 0) & (k_sel < NK)), "idx_identit outside its block"
    u = np.zeros((C_OUT, NK), np.float32)
    for g in range(G):
        np.add.at(u, (np.arange(C_OUT), k_sel[:, g]), w3r[g])
    co_i, k_i = np.nonzero(u != 0)
    for co, k in zip(co_i, k_i):
        opv[co, k * HIN + pos + EP, 64 + pos] += u[co, k]
    return opv, oph


def _chunk_ops(op):
    """(96, 660, M) -> (96, KCH, NCHUNK, M) bf16: chunk rows, partition-major."""
    m = op.shape[2]
    # (96, NCHUNK, KCH, m) -> (96, KCH, NCHUNK, m)
    r = op.reshape(C_OUT, NCHUNK, KCH, m).transpose(0, 2, 1, 3)
    return np.ascontiguousarray(r.astype(BF16))


def _build_nc():
    import concourse.bacc as bacc
    import concourse.tile as tile
    import concourse.bass as bass
    import concourse.mybir as mybir
    from contextlib import ExitStack

    f32 = mybir.dt.float32
    bf16 = mybir.dt.bfloat16

    nc = bacc.Bacc(None, target_bir_lowering=False)
    xv_d = nc.declare_dram_parameter("xv", [CPC * HIN, NFREE], bf16, isOutput=False)
    xh_d = nc.declare_dram_parameter("xh", [CPC * WIN, NFREE], bf16, isOutput=False)
    opv_d = nc.declare_dram_parameter("opv", [KCH, BPC, NCHUNK, 120], bf16, isOutput=False)
    oph_d = nc.declare_dram_parameter("oph", [KCH, BPC, NCHUNK, 56], bf16, isOutput=False)
    out_d = nc.declare_dram_parameter("out", [BPC, 3, 56, NFREE], f32, isOutput=True)

    with tile.TileContext(nc) as tc, ExitStack() as ctx:
        rhs_pool = ctx.enter_context(tc.tile_pool(name="rhs", bufs=3))
        op_pool = ctx.enter_context(tc.tile_pool(name="ops", bufs=1))
        o_pool = ctx.enter_context(tc.tile_pool(name="outs", bufs=3))
        psum_pool = ctx.enter_context(
            tc.tile_pool(name="psum", bufs=2, space=bass.MemorySpace.PSUM)
        )
        # all operators resident up-front (two large DMAs)
        opv_t = op_pool.tile([KCH, BPC, NCHUNK, 120], bf16, tag="opv")
        nc.sync.dma_start(opv_t[:], opv_d[:])
        oph_t = op_pool.tile([KCH, BPC, NCHUNK, 56], bf16, tag="oph")
        nc.sync.dma_start(oph_t[:], oph_d[:])

        for bi in range(BPC):
            # one 633KB DMA per (block, orientation): rows (j p) f -> p j f
            rv = rhs_pool.tile([KCH, NCHUNK, NFREE], bf16, tag="rv")
            nc.sync.dma_start(
                rv[:],
                xv_d[bi * KROWS:(bi + 1) * KROWS, :].rearrange(
                    "(j p) f -> p j f", j=NCHUNK),
            )
            psum_vi = psum_pool.tile([120, NFREE], f32, tag="pv")
            for j in range(NCHUNK):
                nc.tensor.matmul(
                    psum_vi[:], opv_t[:, bi, j, :], rv[:, j, :],
                    start=(j == 0), stop=(j == NCHUNK - 1),
                )
            rh = rhs_pool.tile([KCH, NCHUNK, NFREE], bf16, tag="rh")
            nc.sync.dma_start(
                rh[:],
                xh_d[bi * KROWS:(bi + 1) * KROWS, :].rearrange(
                    "(j p) f -> p j f", j=NCHUNK),
            )
            psum_h = psum_pool.tile([56, NFREE], f32, tag="ph")
            for j in range(NCHUNK):
                nc.tensor.matmul(
                    psum_h[:], oph_t[:, bi, j, :], rh[:, j, :],
                    start=(j == 0), stop=(j == NCHUNK - 1),
                )
            # stage [56, 3, 480]: slot 0 = V, 1 = I, 2 = H; one DMA out
            st = o_pool.tile([56, 3, NFREE], f32, tag="st")
            nc.scalar.copy(st[:, 0, :], psum_vi[:56])
            nc.vector.tensor_copy(st[:, 1, :], psum_vi[64:120])
            nc.vector.tensor_copy(st[:, 2, :], psum_h[:])
            nc.sync.dma_start(out_d[bi].rearrange("t h f -> h t f"), st[:])
    nc.finalize()
    return nc


def prepare_inputs(x, w1, w2, w3, pad_hv, idx_identit):
    """Host-side shard prep. Returns in_maps (list of 8 dicts)."""
    x = np.asarray(x)
    xb = x.astype(BF16)                                   # (B, C, 60, 60)
    # h-major for V/I: [c, h, (b, w)]
    x_hbw = np.ascontiguousarray(xb.transpose(1, 2, 0, 3)).reshape(C_IN * HIN, NFREE)
    # w-major for H: [c, w, (b, h)]
    x_wbh = np.ascontiguousarray(xb.transpose(1, 3, 0, 2)).reshape(C_IN * WIN, NFREE)

    opv, oph = _build_operators(w1, w2, w3, pad_hv, idx_identit)
    opv_c = _chunk_ops(opv)                                # (96, 128, 6, 112)
    oph_c = _chunk_ops(oph)                                # (96, 128, 6, 56)

    in_maps = []
    for i in range(N_CORES):
        r0 = i * CPC * HIN
        in_maps.append({
            "xv": x_hbw[r0:r0 + CPC * HIN],
            "xh": x_wbh[r0:r0 + CPC * WIN],
            # (BPC, KCH, NCHUNK, M) -> (KCH, BPC, NCHUNK, M)
            "opv": np.ascontiguousarray(
                opv_c[i * BPC:(i + 1) * BPC].transpose(1, 0, 2, 3)),
            "oph": np.ascontiguousarray(
                oph_c[i * BPC:(i + 1) * BPC].transpose(1, 0, 2, 3)),
        })
    return in_maps


def unshard(results):
    """results: list of 8 dicts with 'out' (BPC, 3, 56, 480) fp32 ->
    (out_h, out_v, out_i) each (B, C_OUT, 56, 56) fp32."""
    O = np.stack([np.asarray(r["out"], np.float32) for r in results])  # (8,12,3,56,480)
    O = O.reshape(N_CORES, BPC, 3, 56, B, 60)
    vi = O[:, :, 0:2, :, :, EP:EP + WOUT]     # (core, co_l, 2, h, b, w)
    # (core, co_l, h, b, w) -> (b, core, co_l, h, w)
    out_v = vi[:, :, 0].transpose(3, 0, 1, 2, 4).reshape(B, C_OUT, HOUT, WOUT)
    out_i = vi[:, :, 1].transpose(3, 0, 1, 2, 4).reshape(B, C_OUT, HOUT, WOUT)
    h = O[:, :, 2, :, :, EP:EP + HOUT]        # (core, co_l, w, b, h)
    out_h = h.transpose(3, 0, 1, 4, 2).reshape(B, C_OUT, HOUT, WOUT)
    return out_h, out_v, out_i


def kernel(x, w1, w2, w3, pad_hv, idx_identit, b=B, hout=HOUT, wout=WOUT):
    from concourse.bass_utils import run_bass_kernel_spmd

    assert int(b) == B and int(hout) == HOUT and int(wout) == WOUT
    assert tuple(np.asarray(x).shape) == (B, C_IN, HIN, WIN)

    in_maps = prepare_inputs(x, w1, w2, w3, pad_hv, idx_identit)
    nc = _CACHE.get("nc")
    if nc is None:
        nc = _build_nc()
        _CACHE["nc"] = nc
    res = run_bass_kernel_spmd(nc, in_maps, core_ids=list(range(N_CORES)))
    return unshard(res.results)


# revision 9
# speedup vs baseline: 1.4930x; 1.2219x over previous
"""AddShift_mp_linear_module on 8 TRN2 NeuronCores.

Strategy (channel-block sharding, no collectives):
  - 96 output-channel blocks (11 input channels each) -> 12 blocks/core.
  - Every branch is a contraction over the block's (k, spatial) axis:
      out_v[co, h, (b,w)]  = sum_{k,h'} Ov[(k,h'), h]   * x[b, c, h', w]
      out_i[co, h, (b,w)]  = sum_{k,h'} Oi[(k,h'), h]   * x[b, c, h', w]
      out_h[co, w, (b,h')] = sum_{k,w'} Oh[(k,w'), w]   * x[b, c, h', w']
    where the sparse operators Ov/Oi/Oh are built on the host from
    w1/w2/w3/pad_hv/idx_identit (all known at call time).
  - On device: per block, 6 PSUM-accumulated matmuls over K-chunks of the
    660-row contraction (5x128 + 20). V and I share one matmul chain
    (stationary [K,112] = [V | I] columns). H uses a host-pretransposed
    copy of x (w-major) since the tensor engine contracts partitions.
  - x is cast to bf16 on host; batch rides in the matmul free dim (8*60=480).
  - Outputs return as [56, 480] fp32 tiles; host restores [b, co, h, w].
"""

import numpy as np
import ml_dtypes

# architecture constants (match reference init_kwargs)
B = 8
C_OUT = 96
NK = 11
G = 4
C_IN = C_OUT * NK          # 1056
HOUT = WOUT = 56
HIN = WIN = 60
EP = 2                     # extra pad
N_CORES = 8
BPC = C_OUT // N_CORES     # blocks per core = 12
CPC = BPC * NK             # channels per core = 132
KROWS = NK * HIN           # 660 contraction rows per block
KCH = 110                  # K-chunk size (660 = 6*110, uniform)
NCHUNK = KROWS // KCH      # 6
NFREE = B * WIN            # 480 matmul free dim

BF16 = ml_dtypes.bfloat16

_CACHE = {}


def _build_operators(w1, w2, w3, pad_hv, idx_identit):
    """Build per-block stationary operators.

    Returns opv (96, 660, 120) fp32  [cols 0:56 = V, 64:120 = identity]
            oph (96, 660, 56)  fp32
    Row r = k*60 + spatial_in, for channel c = co*11 + k.
    """
    w1r = np.asarray(w1, np.float32).reshape(G, C_IN)
    w2r = np.asarray(w2, np.float32).reshape(G, C_IN)
    w3r = np.asarray(w3, np.float32).reshape(G, C_OUT)
    pad = np.asarray(pad_hv, np.int64)            # (C_IN, 2G)
    idx = np.asarray(idx_identit, np.int64)       # (C_OUT, G)

    opv = np.zeros((C_OUT, KROWS, 120), np.float32)
    oph = np.zeros((C_OUT, KROWS, 56), np.float32)

    c_all = np.arange(C_IN)
    co_all = c_all // NK
    k_all = c_all % NK
    pos = np.arange(HOUT)                          # output spatial index

    for g in range(G):
        # horizontal: w_in = w_out + EP + pad[c, g]
        win = pos[None, :] + EP + pad[:, g][:, None]        # (C_IN, 56)
        ok = (win >= 0) & (win < WIN)
        cc, oo = np.nonzero(ok)
        np.add.at(oph, (co_all[cc], k_all[cc] * HIN + win[cc, oo], oo), w1r[g, cc])
        # vertical: h_in = h_out + EP + pad[c, G+g]
        hin = pos[None, :] + EP + pad[:, G + g][:, None]
        ok = (hin >= 0) & (hin < HIN)
        cc, oo = np.nonzero(ok)
        np.add.at(opv, (co_all[cc], k_all[cc] * HIN + hin[cc, oo], oo), w2r[g, cc])

    # identity: out_i[co] = sum_g w3r[g, co] * x[idx[co, g]] (idx within block co)
    k_sel = idx - np.arange(C_OUT)[:, None] * NK            # (C_OUT, G)
    assert np.all((k_sel >= 0) & (k_sel < NK)), "idx_identit outside its block"
    u = np.zeros((C_OUT, NK), np.float32)
    for g in range(G):
        np.add.at(u, (np.arange(C_OUT), k_sel[:, g]), w3r[g])
    co_i, k_i = np.nonzero(u != 0)
    for co, k in zip(co_i, k_i):
        opv[co, k * HIN + pos + EP, 64 + pos] += u[co, k]
    return opv, oph


def _chunk_ops(op):
    """(96, 660, M) -> (96, KCH, NCHUNK, M) bf16: chunk rows, partition-major."""
    m = op.shape[2]
    # (96, NCHUNK, KCH, m) -> (96, KCH, NCHUNK, m)
    r = op.reshape(C_OUT, NCHUNK, KCH, m).transpose(0, 2, 1, 3)
    return np.ascontiguousarray(r.astype(BF16))


def _build_nc():
    import concourse.bacc as bacc
    import concourse.tile as tile
    import concourse.bass as bass
    import concourse.mybir as mybir
    from contextlib import ExitStack

    f32 = mybir.dt.float32
    bf16 = mybir.dt.bfloat16

    nc = bacc.Bacc(None, target_bir_lowering=False)
    # x in both orientations, pre-arranged to the exact SBUF layout:
    # xall[bi, p, s, j, f]  s=0: h-major rows (V/I), s=1: w-major rows (H)
    xall_d = nc.declare_dram_parameter(
        "xall", [BPC, KCH, 2, NCHUNK, NFREE], bf16, isOutput=False)
    opv_d = nc.declare_dram_parameter("opv", [KCH, BPC, NCHUNK, 120], bf16, isOutput=False)
    oph_d = nc.declare_dram_parameter("oph", [KCH, BPC, NCHUNK, 56], bf16, isOutput=False)
    out_d = nc.declare_dram_parameter("out", [BPC, 56, 3, NFREE], bf16, isOutput=True)

    with tile.TileContext(nc) as tc, ExitStack() as ctx:
        rhs_pool = ctx.enter_context(tc.tile_pool(name="rhs", bufs=3))
        op_pool = ctx.enter_context(tc.tile_pool(name="ops", bufs=1))
        o_pool = ctx.enter_context(tc.tile_pool(name="outs", bufs=3))
        psum_pool = ctx.enter_context(
            tc.tile_pool(name="psum", bufs=2, space=bass.MemorySpace.PSUM)
        )
        # all operators resident up-front (two large contiguous DMAs)
        opv_t = op_pool.tile([KCH, BPC, NCHUNK, 120], bf16, tag="opv")
        nc.sync.dma_start(opv_t[:], opv_d[:])
        oph_t = op_pool.tile([KCH, BPC, NCHUNK, 56], bf16, tag="oph")
        nc.scalar.dma_start(oph_t[:], oph_d[:])

        for bi in range(BPC):
            # one 1.27MB DMA per block: contiguous 11.5KB per partition
            xt = rhs_pool.tile([KCH, 2, NCHUNK, NFREE], bf16, tag="xt")
            (nc.sync if bi % 2 == 0 else nc.scalar).dma_start(xt[:], xall_d[bi])
            psum_vi = psum_pool.tile([120, NFREE], f32, tag="pv")
            for j in range(NCHUNK):
                nc.tensor.matmul(
                    psum_vi[:], opv_t[:, bi, j, :], xt[:, 0, j, :],
                    start=(j == 0), stop=(j == NCHUNK - 1),
                )
            psum_h = psum_pool.tile([56, NFREE], f32, tag="ph")
            for j in range(NCHUNK):
                nc.tensor.matmul(
                    psum_h[:], oph_t[:, bi, j, :], xt[:, 1, j, :],
                    start=(j == 0), stop=(j == NCHUNK - 1),
                )
            # stage [56, (3, 480)] bf16: slot 0 = V, 1 = I, 2 = H; one DMA out
            st = o_pool.tile([56, 3, NFREE], bf16, tag="st")
            nc.scalar.copy(st[:, 0, :], psum_vi[:56])
            nc.vector.tensor_copy(st[:, 1, :], psum_vi[64:120])
            nc.vector.tensor_copy(st[:, 2, :], psum_h[:])
            (nc.scalar if bi % 2 == 0 else nc.sync).dma_start(out_d[bi], st[:])
    nc.finalize()
    return nc


def prepare_inputs(x, w1, w2, w3, pad_hv, idx_identit):
    """Host-side shard prep. Returns in_maps (list of 8 dicts)."""
    x = np.asarray(x)
    xb = x.astype(BF16)                                   # (B, C, 60, 60)
    # h-major for V/I: [c, h, (b, w)]
    x_hbw = np.ascontiguousarray(xb.transpose(1, 2, 0, 3)).reshape(C_IN * HIN, NFREE)
    # w-major for H: [c, w, (b, h)]
    x_wbh = np.ascontiguousarray(xb.transpose(1, 3, 0, 2)).reshape(C_IN * WIN, NFREE)

    opv, oph = _build_operators(w1, w2, w3, pad_hv, idx_identit)
    opv_c = _chunk_ops(opv)                                # (96, 128, 6, 112)
    oph_c = _chunk_ops(oph)                                # (96, 128, 6, 56)

    in_maps = []
    for i in range(N_CORES):
        r0 = i * CPC * HIN
        # rows (bi, j, p) -> (bi, p, j): per-partition contiguous chunks
        xv_r = x_hbw[r0:r0 + CPC * HIN].reshape(BPC, NCHUNK, KCH, NFREE)
        xh_r = x_wbh[r0:r0 + CPC * WIN].reshape(BPC, NCHUNK, KCH, NFREE)
        xall = np.stack([xv_r, xh_r], axis=3)          # (BPC, NCHUNK, KCH, 2, F)
        xall = xall.transpose(0, 2, 3, 1, 4)           # (BPC, KCH, 2, NCHUNK, F)
        in_maps.append({
            "xall": np.ascontiguousarray(xall),
            # (BPC, KCH, NCHUNK, M) -> (KCH, BPC, NCHUNK, M)
            "opv": np.ascontiguousarray(
                opv_c[i * BPC:(i + 1) * BPC].transpose(1, 0, 2, 3)),
            "oph": np.ascontiguousarray(
                oph_c[i * BPC:(i + 1) * BPC].transpose(1, 0, 2, 3)),
        })
    return in_maps


def unshard(results):
    """results: list of 8 dicts with 'out' (BPC, 3, 56, 480) fp32 ->
    (out_h, out_v, out_i) each (B, C_OUT, 56, 56) fp32."""
    O = np.stack([np.asarray(r["out"], np.float32) for r in results])  # (8,12,56,3,480)
    O = O.reshape(N_CORES, BPC, 56, 3, B, 60)
    vi = O[:, :, :, 0:2, :, EP:EP + WOUT]     # (core, co_l, h, 2, b, w)
    # (core, co_l, h, b, w) -> (b, core, co_l, h, w)
    out_v = vi[:, :, :, 0].transpose(3, 0, 1, 2, 4).reshape(B, C_OUT, HOUT, WOUT)
    out_i = vi[:, :, :, 1].transpose(3, 0, 1, 2, 4).reshape(B, C_OUT, HOUT, WOUT)
    h = O[:, :, :, 2, :, EP:EP + HOUT]        # (core, co_l, w, b, h)
    out_h = h.transpose(3, 0, 1, 4, 2).reshape(B, C_OUT, HOUT, WOUT)
    return out_h, out_v, out_i


def kernel(x, w1, w2, w3, pad_hv, idx_identit, b=B, hout=HOUT, wout=WOUT):
    from concourse.bass_utils import run_bass_kernel_spmd

    assert int(b) == B and int(hout) == HOUT and int(wout) == WOUT
    assert tuple(np.asarray(x).shape) == (B, C_IN, HIN, WIN)

    in_maps = prepare_inputs(x, w1, w2, w3, pad_hv, idx_identit)
    nc = _CACHE.get("nc")
    if nc is None:
        nc = _build_nc()
        _CACHE["nc"] = nc
    res = run_bass_kernel_spmd(nc, in_maps, core_ids=list(range(N_CORES)))
    return unshard(res.results)


# revision 10
# speedup vs baseline: 1.5524x; 1.0398x over previous
"""AddShift_mp_linear_module on 8 TRN2 NeuronCores.

Strategy (channel-block sharding, no collectives):
  - 96 output-channel blocks (11 input channels each) -> 12 blocks/core.
  - Every branch is a contraction over the block's (k, spatial) axis:
      out_v[co, h, (b,w)]  = sum_{k,h'} Ov[(k,h'), h]   * x[b, c, h', w]
      out_i[co, h, (b,w)]  = sum_{k,h'} Oi[(k,h'), h]   * x[b, c, h', w]
      out_h[co, w, (b,h')] = sum_{k,w'} Oh[(k,w'), w]   * x[b, c, h', w']
    where the sparse operators Ov/Oi/Oh are built on the host from
    w1/w2/w3/pad_hv/idx_identit (all known at call time).
  - On device: per block, 6 PSUM-accumulated matmuls over K-chunks of the
    660-row contraction (5x128 + 20). V and I share one matmul chain
    (stationary [K,112] = [V | I] columns). H uses a host-pretransposed
    copy of x (w-major) since the tensor engine contracts partitions.
  - x is cast to bf16 on host; batch rides in the matmul free dim (8*60=480).
  - Outputs return as [56, 480] fp32 tiles; host restores [b, co, h, w].
"""

import numpy as np
import ml_dtypes

# architecture constants (match reference init_kwargs)
B = 8
C_OUT = 96
NK = 11
G = 4
C_IN = C_OUT * NK          # 1056
HOUT = WOUT = 56
HIN = WIN = 60
EP = 2                     # extra pad
N_CORES = 8
BPC = C_OUT // N_CORES     # blocks per core = 12
CPC = BPC * NK             # channels per core = 132
KROWS = NK * HIN           # 660 contraction rows per block
KCH = 110                  # K-chunk size (660 = 6*110, uniform)
NCHUNK = KROWS // KCH      # 6
NFREE = B * WIN            # 480 matmul free dim

BF16 = ml_dtypes.bfloat16

_CACHE = {}


def _build_operators(w1, w2, w3, pad_hv, idx_identit):
    """Build per-block stationary operators.

    Returns opv (96, 660, 120) fp32  [cols 0:56 = V, 64:120 = identity]
            oph (96, 660, 56)  fp32
    Row r = k*60 + spatial_in, for channel c = co*11 + k.
    """
    w1r = np.asarray(w1, np.float32).reshape(G, C_IN)
    w2r = np.asarray(w2, np.float32).reshape(G, C_IN)
    w3r = np.asarray(w3, np.float32).reshape(G, C_OUT)
    pad = np.asarray(pad_hv, np.int64)            # (C_IN, 2G)
    idx = np.asarray(idx_identit, np.int64)       # (C_OUT, G)

    opv = np.zeros((C_OUT, KROWS, 120), np.float32)
    oph = np.zeros((C_OUT, KROWS, 56), np.float32)

    c_all = np.arange(C_IN)
    co_all = c_all // NK
    k_all = c_all % NK
    pos = np.arange(HOUT)                          # output spatial index

    for g in range(G):
        # horizontal: w_in = w_out + EP + pad[c, g]
        win = pos[None, :] + EP + pad[:, g][:, None]        # (C_IN, 56)
        ok = (win >= 0) & (win < WIN)
        cc, oo = np.nonzero(ok)
        np.add.at(oph, (co_all[cc], k_all[cc] * HIN + win[cc, oo], oo), w1r[g, cc])
        # vertical: h_in = h_out + EP + pad[c, G+g]
        hin = pos[None, :] + EP + pad[:, G + g][:, None]
        ok = (hin >= 0) & (hin < HIN)
        cc, oo = np.nonzero(ok)
        np.add.at(opv, (co_all[cc], k_all[cc] * HIN + hin[cc, oo], oo), w2r[g, cc])

    # identity: out_i[co] = sum_g w3r[g, co] * x[idx[co, g]] (idx within block co)
    k_sel = idx - np.arange(C_OUT)[:, None] * NK            # (C_OUT, G)
    assert np.all((k_sel >= 0) & (k_sel < NK)), "idx_identit outside its block"
    u = np.zeros((C_OUT, NK), np.float32)
    for g in range(G):
        np.add.at(u, (np.arange(C_OUT), k_sel[:, g]), w3r[g])
    co_i, k_i = np.nonzero(u != 0)
    for co, k in zip(co_i, k_i):
        opv[co, k * HIN + pos + EP, 64 + pos] += u[co, k]
    return opv, oph


def _chunk_ops(op):
    """(96, 660, M) -> (96, KCH, NCHUNK, M) bf16: chunk rows, partition-major."""
    m = op.shape[2]
    # (96, NCHUNK, KCH, m) -> (96, KCH, NCHUNK, m)
    r = op.reshape(C_OUT, NCHUNK, KCH, m).transpose(0, 2, 1, 3)
    return np.ascontiguousarray(r.astype(BF16))


def _build_nc():
    import concourse.bacc as bacc
    import concourse.tile as tile
    import concourse.bass as bass
    import concourse.mybir as mybir
    from contextlib import ExitStack

    f32 = mybir.dt.float32
    bf16 = mybir.dt.bfloat16

    nc = bacc.Bacc(None, target_bir_lowering=False)
    # x in both orientations, pre-arranged to the exact SBUF layout:
    # xall[bi, p, s, j, f]  s=0: h-major rows (V/I), s=1: w-major rows (H)
    xall_d = nc.declare_dram_parameter(
        "xall", [BPC, KCH, 2, NCHUNK, NFREE], bf16, isOutput=False)
    opv_d = nc.declare_dram_parameter("opv", [KCH, BPC, NCHUNK, 120], bf16, isOutput=False)
    oph_d = nc.declare_dram_parameter("oph", [KCH, BPC, NCHUNK, 56], bf16, isOutput=False)
    out_d = nc.declare_dram_parameter("out", [BPC, 56, 3, NFREE], bf16, isOutput=True)

    with tile.TileContext(nc) as tc, ExitStack() as ctx:
        rhs_pool = ctx.enter_context(tc.tile_pool(name="rhs", bufs=4))
        op_pool = ctx.enter_context(tc.tile_pool(name="ops", bufs=1))
        o_pool = ctx.enter_context(tc.tile_pool(name="outs", bufs=3))
        psum_pool = ctx.enter_context(
            tc.tile_pool(name="psum", bufs=2, space=bass.MemorySpace.PSUM)
        )
        for bi in range(BPC):
            # per-block operator tiles so block 0 can start immediately
            opv_t = op_pool.tile([KCH, NCHUNK, 120], bf16, tag=f"opv{bi}")
            nc.sync.dma_start(opv_t[:], opv_d[:, bi])
            oph_t = op_pool.tile([KCH, NCHUNK, 56], bf16, tag=f"oph{bi}")
            nc.scalar.dma_start(oph_t[:], oph_d[:, bi])
            # one 1.27MB DMA per block: contiguous 11.5KB per partition
            xt = rhs_pool.tile([KCH, 2, NCHUNK, NFREE], bf16, tag="xt")
            (nc.sync if bi % 2 == 0 else nc.scalar).dma_start(xt[:], xall_d[bi])
            psum_vi = psum_pool.tile([120, NFREE], f32, tag="pv")
            for j in range(NCHUNK):
                nc.tensor.matmul(
                    psum_vi[:], opv_t[:, j, :], xt[:, 0, j, :],
                    start=(j == 0), stop=(j == NCHUNK - 1),
                )
            psum_h = psum_pool.tile([56, NFREE], f32, tag="ph")
            for j in range(NCHUNK):
                nc.tensor.matmul(
                    psum_h[:], oph_t[:, j, :], xt[:, 1, j, :],
                    start=(j == 0), stop=(j == NCHUNK - 1),
                )
            # stage [56, (3, 480)] bf16: slot 0 = V, 1 = I, 2 = H; one DMA out
            st = o_pool.tile([56, 3, NFREE], bf16, tag="st")
            nc.scalar.copy(st[:, 0, :], psum_vi[:56])
            nc.vector.tensor_copy(st[:, 1, :], psum_vi[64:120])
            nc.vector.tensor_copy(st[:, 2, :], psum_h[:])
            nc.gpsimd.dma_start(out_d[bi], st[:])
    nc.finalize()
    return nc


def prepare_inputs(x, w1, w2, w3, pad_hv, idx_identit):
    """Host-side shard prep. Returns in_maps (list of 8 dicts)."""
    x = np.asarray(x)
    xb = x.astype(BF16)                                   # (B, C, 60, 60)
    # h-major for V/I: [c, h, (b, w)]
    x_hbw = np.ascontiguousarray(xb.transpose(1, 2, 0, 3)).reshape(C_IN * HIN, NFREE)
    # w-major for H: [c, w, (b, h)]
    x_wbh = np.ascontiguousarray(xb.transpose(1, 3, 0, 2)).reshape(C_IN * WIN, NFREE)

    opv, oph = _build_operators(w1, w2, w3, pad_hv, idx_identit)
    opv_c = _chunk_ops(opv)                                # (96, 128, 6, 112)
    oph_c = _chunk_ops(oph)                                # (96, 128, 6, 56)

    in_maps = []
    for i in range(N_CORES):
        r0 = i * CPC * HIN
        # rows (bi, j, p) -> (bi, p, j): per-partition contiguous chunks
        xv_r = x_hbw[r0:r0 + CPC * HIN].reshape(BPC, NCHUNK, KCH, NFREE)
        xh_r = x_wbh[r0:r0 + CPC * WIN].reshape(BPC, NCHUNK, KCH, NFREE)
        xall = np.stack([xv_r, xh_r], axis=3)          # (BPC, NCHUNK, KCH, 2, F)
        xall = xall.transpose(0, 2, 3, 1, 4)           # (BPC, KCH, 2, NCHUNK, F)
        in_maps.append({
            "xall": np.ascontiguousarray(xall),
            # (BPC, KCH, NCHUNK, M) -> (KCH, BPC, NCHUNK, M)
            "opv": np.ascontiguousarray(
                opv_c[i * BPC:(i + 1) * BPC].transpose(1, 0, 2, 3)),
            "oph": np.ascontiguousarray(
                oph_c[i * BPC:(i + 1) * BPC].transpose(1, 0, 2, 3)),
        })
    return in_maps


def unshard(results):
    """results: list of 8 dicts with 'out' (BPC, 3, 56, 480) fp32 ->
    (out_h, out_v, out_i) each (B, C_OUT, 56, 56) fp32."""
    O = np.stack([np.asarray(r["out"], np.float32) for r in results])  # (8,12,56,3,480)
    O = O.reshape(N_CORES, BPC, 56, 3, B, 60)
    vi = O[:, :, :, 0:2, :, EP:EP + WOUT]     # (core, co_l, h, 2, b, w)
    # (core, co_l, h, b, w) -> (b, core, co_l, h, w)
    out_v = vi[:, :, :, 0].transpose(3, 0, 1, 2, 4).reshape(B, C_OUT, HOUT, WOUT)
    out_i = vi[:, :, :, 1].transpose(3, 0, 1, 2, 4).reshape(B, C_OUT, HOUT, WOUT)
    h = O[:, :, :, 2, :, EP:EP + HOUT]        # (core, co_l, w, b, h)
    out_h = h.transpose(3, 0, 1, 4, 2).reshape(B, C_OUT, HOUT, WOUT)
    return out_h, out_v, out_i


def kernel(x, w1, w2, w3, pad_hv, idx_identit, b=B, hout=HOUT, wout=WOUT):
    from concourse.bass_utils import run_bass_kernel_spmd

    assert int(b) == B and int(hout) == HOUT and int(wout) == WOUT
    assert tuple(np.asarray(x).shape) == (B, C_IN, HIN, WIN)

    in_maps = prepare_inputs(x, w1, w2, w3, pad_hv, idx_identit)
    nc = _CACHE.get("nc")
    if nc is None:
        nc = _build_nc()
        _CACHE["nc"] = nc
    res = run_bass_kernel_spmd(nc, in_maps, core_ids=list(range(N_CORES)))
    return unshard(res.results)


# revision 11
# speedup vs baseline: 1.8560x; 1.1956x over previous
"""AddShift_mp_linear_module on 8 TRN2 NeuronCores.

Strategy (channel-block sharding, no collectives):
  - 96 output-channel blocks (11 input channels each) -> 12 blocks/core.
  - Every branch is a contraction over the block's (k, spatial) axis:
      out_v[co, h, (b,w)]  = sum_{k,h'} Ov[(k,h'), h]   * x[b, c, h', w]
      out_i[co, h, (b,w)]  = sum_{k,h'} Oi[(k,h'), h]   * x[b, c, h', w]
      out_h[co, w, (b,h')] = sum_{k,w'} Oh[(k,w'), w]   * x[b, c, h', w']
    where the sparse operators Ov/Oi/Oh are built on the host from
    w1/w2/w3/pad_hv/idx_identit (all known at call time).
  - On device: per block, 6 PSUM-accumulated matmuls over K-chunks of the
    660-row contraction (5x128 + 20). V and I share one matmul chain
    (stationary [K,112] = [V | I] columns). H uses a host-pretransposed
    copy of x (w-major) since the tensor engine contracts partitions.
  - x is cast to bf16 on host; batch rides in the matmul free dim (8*60=480).
  - Outputs return as [56, 480] fp32 tiles; host restores [b, co, h, w].
"""

import numpy as np
import ml_dtypes

# architecture constants (match reference init_kwargs)
B = 8
C_OUT = 96
NK = 11
G = 4
C_IN = C_OUT * NK          # 1056
HOUT = WOUT = 56
HIN = WIN = 60
EP = 2                     # extra pad
N_CORES = 8
BPC = C_OUT // N_CORES     # blocks per core = 12
CPC = BPC * NK             # channels per core = 132
KROWS = NK * HIN           # 660 real contraction rows per block
KCH = 128                  # K-chunk size (padded: 6*128 = 768 rows/block)
NCHUNK = 6
KPAD = KCH * NCHUNK        # 768
NFREE = B * WIN            # 480 matmul free dim

BF16 = ml_dtypes.bfloat16

_CACHE = {}


def _build_operators(w1, w2, w3, pad_hv, idx_identit):
    """Build per-block stationary operators.

    Returns opv (96, 660, 120) fp32  [cols 0:56 = V, 64:120 = identity]
            oph (96, 660, 56)  fp32
    Row r = k*60 + spatial_in, for channel c = co*11 + k.
    """
    w1r = np.asarray(w1, np.float32).reshape(G, C_IN)
    w2r = np.asarray(w2, np.float32).reshape(G, C_IN)
    w3r = np.asarray(w3, np.float32).reshape(G, C_OUT)
    pad = np.asarray(pad_hv, np.int64)            # (C_IN, 2G)
    idx = np.asarray(idx_identit, np.int64)       # (C_OUT, G)

    opv = np.zeros((C_OUT, KROWS, 120), np.float32)
    oph = np.zeros((C_OUT, KROWS, 56), np.float32)

    c_all = np.arange(C_IN)
    co_all = c_all // NK
    k_all = c_all % NK
    pos = np.arange(HOUT)                          # output spatial index

    for g in range(G):
        # horizontal: w_in = w_out + EP + pad[c, g]
        win = pos[None, :] + EP + pad[:, g][:, None]        # (C_IN, 56)
        ok = (win >= 0) & (win < WIN)
        cc, oo = np.nonzero(ok)
        np.add.at(oph, (co_all[cc], k_all[cc] * HIN + win[cc, oo], oo), w1r[g, cc])
        # vertical: h_in = h_out + EP + pad[c, G+g]
        hin = pos[None, :] + EP + pad[:, G + g][:, None]
        ok = (hin >= 0) & (hin < HIN)
        cc, oo = np.nonzero(ok)
        np.add.at(opv, (co_all[cc], k_all[cc] * HIN + hin[cc, oo], oo), w2r[g, cc])

    # identity: out_i[co] = sum_g w3r[g, co] * x[idx[co, g]] (idx within block co)
    k_sel = idx - np.arange(C_OUT)[:, None] * NK            # (C_OUT, G)
    assert np.all((k_sel >= 0) & (k_sel < NK)), "idx_identit outside its block"
    u = np.zeros((C_OUT, NK), np.float32)
    for g in range(G):
        np.add.at(u, (np.arange(C_OUT), k_sel[:, g]), w3r[g])
    co_i, k_i = np.nonzero(u != 0)
    for co, k in zip(co_i, k_i):
        opv[co, k * HIN + pos + EP, 64 + pos] += u[co, k]
    return opv, oph


def _chunk_ops(op):
    """(96, 660, M) -> (96, KCH, NCHUNK, M) bf16: zero-pad rows to 768, chunk."""
    m = op.shape[2]
    p = np.zeros((C_OUT, KPAD, m), np.float32)
    p[:, :KROWS] = op
    # (96, NCHUNK, KCH, m) -> (96, KCH, NCHUNK, m)
    r = p.reshape(C_OUT, NCHUNK, KCH, m).transpose(0, 2, 1, 3)
    return np.ascontiguousarray(r.astype(BF16))


def _build_nc():
    import concourse.bacc as bacc
    import concourse.tile as tile
    import concourse.bass as bass
    import concourse.mybir as mybir
    from contextlib import ExitStack

    f32 = mybir.dt.float32
    bf16 = mybir.dt.bfloat16

    nc = bacc.Bacc(None, target_bir_lowering=False)
    # x in both orientations, pre-arranged to the exact SBUF layout:
    # xall[bi, p, s, j, f]  s=0: h-major rows (V/I), s=1: w-major rows (H)
    xall_d = nc.declare_dram_parameter(
        "xall", [BPC, KCH, 2, NCHUNK, NFREE], bf16, isOutput=False)
    opv_d = nc.declare_dram_parameter("opv", [KCH, BPC, NCHUNK, 120], bf16, isOutput=False)
    oph_d = nc.declare_dram_parameter("oph", [KCH, BPC, NCHUNK, 56], bf16, isOutput=False)
    out_d = nc.declare_dram_parameter("out", [BPC, 56, 3, NFREE], bf16, isOutput=True)

    with tile.TileContext(nc) as tc, ExitStack() as ctx:
        rhs_pool = ctx.enter_context(tc.tile_pool(name="rhs", bufs=4))
        op_pool = ctx.enter_context(tc.tile_pool(name="ops", bufs=1))
        o_pool = ctx.enter_context(tc.tile_pool(name="outs", bufs=3))
        psum_pool = ctx.enter_context(
            tc.tile_pool(name="psum", bufs=4, space=bass.MemorySpace.PSUM)
        )
        for bi in range(BPC):
            # per-block operator tiles so block 0 can start immediately
            opv_t = op_pool.tile([KCH, NCHUNK, 120], bf16, tag=f"opv{bi}")
            nc.sync.dma_start(opv_t[:], opv_d[:, bi])
            oph_t = op_pool.tile([KCH, NCHUNK, 56], bf16, tag=f"oph{bi}")
            nc.scalar.dma_start(oph_t[:], oph_d[:, bi])
            # one 1.27MB DMA per block: contiguous 11.5KB per partition
            xt = rhs_pool.tile([KCH, 2, NCHUNK, NFREE], bf16, tag="xt")
            (nc.sync if bi % 2 == 0 else nc.scalar).dma_start(xt[:], xall_d[bi])
            psum_vi = psum_pool.tile([120, NFREE], f32, tag="pv")
            for j in range(NCHUNK):
                nc.tensor.matmul(
                    psum_vi[:], opv_t[:, j, :], xt[:, 0, j, :],
                    start=(j == 0), stop=(j == NCHUNK - 1),
                )
            psum_h = psum_pool.tile([56, NFREE], f32, tag="ph")
            for j in range(NCHUNK):
                nc.tensor.matmul(
                    psum_h[:], oph_t[:, j, :], xt[:, 1, j, :],
                    start=(j == 0), stop=(j == NCHUNK - 1),
                )
            # stage [56, (3, 480)] bf16: slot 0 = V, 1 = I, 2 = H; one DMA out
            st = o_pool.tile([56, 3, NFREE], bf16, tag="st")
            nc.scalar.copy(st[:, 0, :], psum_vi[:56])
            nc.vector.tensor_copy(st[:, 1, :], psum_vi[64:120])
            nc.vector.tensor_copy(st[:, 2, :], psum_h[:])
            nc.gpsimd.dma_start(out_d[bi], st[:])
    nc.finalize()
    return nc


def prepare_inputs(x, w1, w2, w3, pad_hv, idx_identit):
    """Host-side shard prep. Returns in_maps (list of 8 dicts)."""
    x = np.asarray(x)
    xb = x.astype(BF16)                                   # (B, C, 60, 60)
    # h-major for V/I: [c, h, (b, w)]
    x_hbw = np.ascontiguousarray(xb.transpose(1, 2, 0, 3)).reshape(C_IN * HIN, NFREE)
    # w-major for H: [c, w, (b, h)]
    x_wbh = np.ascontiguousarray(xb.transpose(1, 3, 0, 2)).reshape(C_IN * WIN, NFREE)

    opv, oph = _build_operators(w1, w2, w3, pad_hv, idx_identit)
    opv_c = _chunk_ops(opv)                                # (96, 128, 6, 112)
    oph_c = _chunk_ops(oph)                                # (96, 128, 6, 56)

    in_maps = []
    for i in range(N_CORES):
        r0 = i * CPC * HIN
        # rows (bi, j, p) -> (bi, p, j), zero-padded 660 -> 768 per block
        xv_r = np.zeros((BPC, KPAD, NFREE), BF16)
        xv_r[:, :KROWS] = x_hbw[r0:r0 + CPC * HIN].reshape(BPC, KROWS, NFREE)
        xh_r = np.zeros((BPC, KPAD, NFREE), BF16)
        xh_r[:, :KROWS] = x_wbh[r0:r0 + CPC * WIN].reshape(BPC, KROWS, NFREE)
        xall = np.stack([xv_r.reshape(BPC, NCHUNK, KCH, NFREE),
                         xh_r.reshape(BPC, NCHUNK, KCH, NFREE)], axis=3)
        xall = xall.transpose(0, 2, 3, 1, 4)           # (BPC, KCH, 2, NCHUNK, F)
        in_maps.append({
            "xall": np.ascontiguousarray(xall),
            # (BPC, KCH, NCHUNK, M) -> (KCH, BPC, NCHUNK, M)
            "opv": np.ascontiguousarray(
                opv_c[i * BPC:(i + 1) * BPC].transpose(1, 0, 2, 3)),
            "oph": np.ascontiguousarray(
                oph_c[i * BPC:(i + 1) * BPC].transpose(1, 0, 2, 3)),
        })
    return in_maps


def unshard(results):
    """results: list of 8 dicts with 'out' (BPC, 3, 56, 480) fp32 ->
    (out_h, out_v, out_i) each (B, C_OUT, 56, 56) fp32."""
    O = np.stack([np.asarray(r["out"], np.float32) for r in results])  # (8,12,56,3,480)
    O = O.reshape(N_CORES, BPC, 56, 3, B, 60)
    vi = O[:, :, :, 0:2, :, EP:EP + WOUT]     # (core, co_l, h, 2, b, w)
    # (core, co_l, h, b, w) -> (b, core, co_l, h, w)
    out_v = vi[:, :, :, 0].transpose(3, 0, 1, 2, 4).reshape(B, C_OUT, HOUT, WOUT)
    out_i = vi[:, :, :, 1].transpose(3, 0, 1, 2, 4).reshape(B, C_OUT, HOUT, WOUT)
    h = O[:, :, :, 2, :, EP:EP + HOUT]        # (core, co_l, w, b, h)
    out_h = h.transpose(3, 0, 1, 4, 2).reshape(B, C_OUT, HOUT, WOUT)
    return out_h, out_v, out_i


def kernel(x, w1, w2, w3, pad_hv, idx_identit, b=B, hout=HOUT, wout=WOUT):
    from concourse.bass_utils import run_bass_kernel_spmd

    assert int(b) == B and int(hout) == HOUT and int(wout) == WOUT
    assert tuple(np.asarray(x).shape) == (B, C_IN, HIN, WIN)

    in_maps = prepare_inputs(x, w1, w2, w3, pad_hv, idx_identit)
    nc = _CACHE.get("nc")
    if nc is None:
        nc = _build_nc()
        _CACHE["nc"] = nc
    res = run_bass_kernel_spmd(nc, in_maps, core_ids=list(range(N_CORES)))
    return unshard(res.results)


# revision 16
# speedup vs baseline: 1.8956x; 1.0213x over previous
"""AddShift_mp_linear_module on 8 TRN2 NeuronCores.

Strategy (channel-block sharding, no collectives):
  - 96 output-channel blocks (11 input channels each) -> 12 blocks/core.
  - Every branch is a contraction over the block's (k, spatial) axis:
      out_v[co, h, (b,w)]  = sum_{k,h'} Ov[(k,h'), h]   * x[b, c, h', w]
      out_i[co, h, (b,w)]  = sum_{k,h'} Oi[(k,h'), h]   * x[b, c, h', w]
      out_h[co, w, (b,h')] = sum_{k,w'} Oh[(k,w'), w]   * x[b, c, h', w']
    where the sparse operators Ov/Oi/Oh are built on the host from
    w1/w2/w3/pad_hv/idx_identit (all known at call time).
  - On device: per block, two interleaved PSUM-accumulation chains of
    matmuls over K-chunks of the 660-row contraction (5x128 + one 20-row
    tail). V and I share one chain (stationary [K,120] = [V | pad | I]
    columns, identity at 64:120 for 32-aligned PSUM reads). H uses a
    host-pretransposed copy of x (w-major) since the PE contracts the
    partition dim. A dummy-matmul warmup burst un-throttles the PE clock
    (HAM) while the first block's DMA is in flight.
  - x is cast to bf16 on host, pre-sliced to the used w/h window and
    pre-arranged so every block loads with one 1.1MB DMA whose per-
    partition runs are contiguous (16 SDMA engines, ~330GB/s). Batch
    rides in the matmul free dim (8*56=448).
  - Outputs leave as [56, 3, 448] bf16 tiles (one DMA per block); host
    restores (out_h, out_v, out_i) [b, co, h, w] fp32.
"""

import numpy as np
import ml_dtypes

# architecture constants (match reference init_kwargs)
B = 8
C_OUT = 96
NK = 11
G = 4
C_IN = C_OUT * NK          # 1056
HOUT = WOUT = 56
HIN = WIN = 60
EP = 2                     # extra pad
N_CORES = 8
BPC = C_OUT // N_CORES     # blocks per core = 12
CPC = BPC * NK             # channels per core = 132
KROWS = NK * HIN           # 660 real contraction rows per block
KCH = 128                  # K-chunk size
NCHUNK = 6                 # 5 full chunks + 20-row tail
KPAD = KCH * NCHUNK        # 768 (operator rows padded; x is not)
KTAIL = KROWS - (NCHUNK - 1) * KCH  # 20
NFREE = B * WOUT           # 448 matmul free dim (w/h pre-sliced to [2,58))

BF16 = ml_dtypes.bfloat16

_CACHE = {}


def _build_operators(w1, w2, w3, pad_hv, idx_identit):
    """Build per-block stationary operators.

    Returns opv (96, 660, 120) fp32  [cols 0:56 = V, 64:120 = identity]
            oph (96, 660, 56)  fp32
    Row r = k*60 + spatial_in, for channel c = co*11 + k.
    """
    w1r = np.asarray(w1, np.float32).reshape(G, C_IN)
    w2r = np.asarray(w2, np.float32).reshape(G, C_IN)
    w3r = np.asarray(w3, np.float32).reshape(G, C_OUT)
    pad = np.asarray(pad_hv, np.int64)            # (C_IN, 2G)
    idx = np.asarray(idx_identit, np.int64)       # (C_OUT, G)

    opv = np.zeros((C_OUT, KROWS, 120), np.float32)
    oph = np.zeros((C_OUT, KROWS, 56), np.float32)

    c_all = np.arange(C_IN)
    co_all = c_all // NK
    k_all = c_all % NK
    pos = np.arange(HOUT)                          # output spatial index

    for g in range(G):
        # horizontal: w_in = w_out + EP + pad[c, g]
        win = pos[None, :] + EP + pad[:, g][:, None]        # (C_IN, 56)
        ok = (win >= 0) & (win < WIN)
        cc, oo = np.nonzero(ok)
        np.add.at(oph, (co_all[cc], k_all[cc] * HIN + win[cc, oo], oo), w1r[g, cc])
        # vertical: h_in = h_out + EP + pad[c, G+g]
        hin = pos[None, :] + EP + pad[:, G + g][:, None]
        ok = (hin >= 0) & (hin < HIN)
        cc, oo = np.nonzero(ok)
        np.add.at(opv, (co_all[cc], k_all[cc] * HIN + hin[cc, oo], oo), w2r[g, cc])

    # identity: out_i[co] = sum_g w3r[g, co] * x[idx[co, g]] (idx within block co)
    k_sel = idx - np.arange(C_OUT)[:, None] * NK            # (C_OUT, G)
    assert np.all((k_sel >= 0) & (k_sel < NK)), "idx_identit outside its block"
    u = np.zeros((C_OUT, NK), np.float32)
    for g in range(G):
        np.add.at(u, (np.arange(C_OUT), k_sel[:, g]), w3r[g])
    co_i, k_i = np.nonzero(u != 0)
    for co, k in zip(co_i, k_i):
        opv[co, k * HIN + pos + EP, 64 + pos] += u[co, k]
    return opv, oph


def _chunk_ops(op):
    """(96, 660, M) -> (96, KCH, NCHUNK, M) bf16: zero-pad rows to 768, chunk."""
    m = op.shape[2]
    p = np.zeros((C_OUT, KPAD, m), np.float32)
    p[:, :KROWS] = op
    # (96, NCHUNK, KCH, m) -> (96, KCH, NCHUNK, m)
    r = p.reshape(C_OUT, NCHUNK, KCH, m).transpose(0, 2, 1, 3)
    return np.ascontiguousarray(r.astype(BF16))


def _build_nc():
    import concourse.bacc as bacc
    import concourse.tile as tile
    import concourse.bass as bass
    import concourse.mybir as mybir
    from contextlib import ExitStack

    f32 = mybir.dt.float32
    bf16 = mybir.dt.bfloat16

    nc = bacc.Bacc(None, target_bir_lowering=False)
    # x in both orientations, pre-arranged to the exact SBUF layout.
    # Main: 5 full 128-row chunks per block; tails (rows 640:660) of all
    # blocks ride in one small resident tile.
    xm_d = nc.declare_dram_parameter(
        "xmain", [BPC, KCH, 2, NCHUNK - 1, NFREE], bf16, isOutput=False)
    # tail rows (640:660): [x_v | x_h | opv | oph] packed on free dim
    TW = 2 * NFREE + 120 + 56                              # 1072
    xt_d = nc.declare_dram_parameter(
        "xtail", [KTAIL, BPC, TW], bf16, isOutput=False)
    opv_d = nc.declare_dram_parameter(
        "opv", [KCH, BPC, NCHUNK - 1, 120], bf16, isOutput=False)
    oph_d = nc.declare_dram_parameter(
        "oph", [KCH, BPC, NCHUNK - 1, 56], bf16, isOutput=False)
    out_d = nc.declare_dram_parameter("out", [BPC, 56, 3, NFREE], bf16, isOutput=True)

    with tile.TileContext(nc) as tc, ExitStack() as ctx:
        rhs_pool = ctx.enter_context(tc.tile_pool(name="rhs", bufs=6))
        op_pool = ctx.enter_context(tc.tile_pool(name="ops", bufs=1))
        o_pool = ctx.enter_context(tc.tile_pool(name="outs", bufs=3))
        psum_pool = ctx.enter_context(
            tc.tile_pool(name="psum", bufs=4, space=bass.MemorySpace.PSUM)
        )
        warmed = False
        for bi in range(BPC):
            # x first (critical path), operators on the opposite HWDGE ring
            xe = nc.sync if bi % 2 == 0 else nc.scalar
            oe = nc.scalar if bi % 2 == 0 else nc.sync
            xt = rhs_pool.tile([KCH, 2, NCHUNK - 1, NFREE], bf16, tag="xt")
            xe.dma_start(xt[:], xm_d[bi])
            opv_t = op_pool.tile([KCH, NCHUNK - 1, 120], bf16, tag=f"opv{bi}")
            oe.dma_start(opv_t[:], opv_d[:, bi])
            oph_t = op_pool.tile([KCH, NCHUNK - 1, 56], bf16, tag=f"oph{bi}")
            oe.dma_start(oph_t[:], oph_d[:, bi])
            if not warmed:
                # HAM warmup: dummy matmuls on the (small, early-arriving)
                # operator tile bridge the gap until x lands, so the PE
                # clock is un-throttled when real work starts
                warmed = True
                ov_flat = opv_t.rearrange("p a m -> p (a m)")
                pw = psum_pool.tile([120, NFREE], f32, tag="ph")
                for _ in range(22):
                    nc.tensor.matmul(
                        pw[:], opv_t[:, 0, :], ov_flat[:, :NFREE],
                        start=True, stop=True)
                # tails for all blocks on the gpsimd ring
                tails = op_pool.tile([KTAIL, BPC, TW], bf16, tag="tails")
                nc.gpsimd.dma_start(tails[:], xt_d[:])
            rv = (lambda xt: lambda j: xt[:, 0, j, :])(xt)
            rh = (lambda xt: lambda j: xt[:, 1, j, :])(xt)
            psum_vi = psum_pool.tile([120, NFREE], f32, tag="pv")
            psum_h = psum_pool.tile([56, NFREE], f32, tag="ph")
            # interleave the two accumulation chains so PE drains overlap
            for j in range(NCHUNK - 1):
                nc.tensor.matmul(
                    psum_vi[:], opv_t[:, j, :], rv(j),
                    start=(j == 0), stop=False,
                )
                nc.tensor.matmul(
                    psum_h[:], oph_t[:, j, :], rh(j),
                    start=(j == 0), stop=False,
                )
            nc.tensor.matmul(
                psum_vi[:], tails[:, bi, 2 * NFREE:2 * NFREE + 120],
                tails[:, bi, :NFREE], start=False, stop=True,
            )
            nc.tensor.matmul(
                psum_h[:], tails[:, bi, 2 * NFREE + 120:],
                tails[:, bi, NFREE:2 * NFREE], start=False, stop=True,
            )
            # stage [56, (3, 448)] bf16: slot 0 = V, 1 = I, 2 = H; one DMA out
            st = o_pool.tile([56, 3, NFREE], bf16, tag="st")
            nc.scalar.copy(st[:, 0, :], psum_vi[:56])
            nc.vector.tensor_copy(st[:, 1, :], psum_vi[64:120])
            nc.vector.tensor_copy(st[:, 2, :], psum_h[:])
            oe.dma_start(out_d[bi], st[:])
    nc.finalize()
    return nc


def prepare_inputs(x, w1, w2, w3, pad_hv, idx_identit):
    """Host-side shard prep. Returns in_maps (list of 8 dicts)."""
    x = np.asarray(x)
    xb = x.astype(BF16)                                   # (B, C, 60, 60)
    # h-major for V/I: [c, h', (b, w in [2,58))]
    x_hbw = np.ascontiguousarray(
        xb[:, :, :, EP:EP + WOUT].transpose(1, 2, 0, 3)).reshape(C_IN * HIN, NFREE)
    # w-major for H: [c, w', (b, h in [2,58))]
    x_wbh = np.ascontiguousarray(
        xb[:, :, EP:EP + HOUT, :].transpose(1, 3, 0, 2)).reshape(C_IN * WIN, NFREE)

    opv, oph = _build_operators(w1, w2, w3, pad_hv, idx_identit)
    opv_c = _chunk_ops(opv)                                # (96, KCH, NCHUNK, 120)
    oph_c = _chunk_ops(oph)                                # (96, KCH, NCHUNK, 56)

    in_maps = []
    for i in range(N_CORES):
        r0 = i * CPC * HIN
        # main: rows 0:640 per block as 5 chunks of 128; tail: rows 640:660
        xv_r = x_hbw[r0:r0 + CPC * HIN].reshape(BPC, KROWS, NFREE)
        xh_r = x_wbh[r0:r0 + CPC * WIN].reshape(BPC, KROWS, NFREE)
        nm = (NCHUNK - 1) * KCH                        # 640
        xmain = np.stack([xv_r[:, :nm].reshape(BPC, NCHUNK - 1, KCH, NFREE),
                          xh_r[:, :nm].reshape(BPC, NCHUNK - 1, KCH, NFREE)],
                         axis=3)                       # (BPC, 5, KCH, 2, F)
        xmain = xmain.transpose(0, 2, 3, 1, 4)         # (BPC, KCH, 2, 5, F)
        # packed tails: [x_v | x_h | opv_tail | oph_tail] -> (20, BPC, 1072)
        ovt = opv_c[i * BPC:(i + 1) * BPC, :KTAIL, NCHUNK - 1, :]  # (BPC,20,120)
        oht = oph_c[i * BPC:(i + 1) * BPC, :KTAIL, NCHUNK - 1, :]  # (BPC,20,56)
        xtail = np.concatenate(
            [xv_r[:, nm:], xh_r[:, nm:], ovt.astype(BF16), oht.astype(BF16)],
            axis=2)                                    # (BPC, 20, 1072)
        xtail = np.ascontiguousarray(xtail.transpose(1, 0, 2))  # (20, BPC, 1072)
        ov = opv_c[i * BPC:(i + 1) * BPC].transpose(1, 0, 2, 3)  # (KCH,BPC,NCHUNK,M)
        oh = oph_c[i * BPC:(i + 1) * BPC].transpose(1, 0, 2, 3)
        in_maps.append({
            "xmain": np.ascontiguousarray(xmain),
            "xtail": xtail,
            "opv": np.ascontiguousarray(ov[:, :, :NCHUNK - 1, :]),
            "oph": np.ascontiguousarray(oh[:, :, :NCHUNK - 1, :]),
        })
    return in_maps


def unshard(results):
    """results: list of 8 dicts with 'out' (BPC, 56, 3, 448) bf16 ->
    (out_h, out_v, out_i) each (B, C_OUT, 56, 56) fp32."""
    O = np.stack([np.asarray(r["out"], np.float32) for r in results])  # (8,12,56,3,448)
    O = O.reshape(N_CORES, BPC, 56, 3, B, WOUT)
    # (core, co_l, h, b, w) -> (b, core, co_l, h, w)
    out_v = O[:, :, :, 0].transpose(3, 0, 1, 2, 4).reshape(B, C_OUT, HOUT, WOUT)
    out_i = O[:, :, :, 1].transpose(3, 0, 1, 2, 4).reshape(B, C_OUT, HOUT, WOUT)
    h = O[:, :, :, 2]                          # (core, co_l, w, b, h)
    out_h = h.transpose(3, 0, 1, 4, 2).reshape(B, C_OUT, HOUT, WOUT)
    return out_h, out_v, out_i


def kernel(x, w1, w2, w3, pad_hv, idx_identit, b=B, hout=HOUT, wout=WOUT):
    from concourse.bass_utils import run_bass_kernel_spmd

    assert int(b) == B and int(hout) == HOUT and int(wout) == WOUT
    assert tuple(np.asarray(x).shape) == (B, C_IN, HIN, WIN)

    in_maps = prepare_inputs(x, w1, w2, w3, pad_hv, idx_identit)
    nc = _CACHE.get("nc")
    if nc is None:
        nc = _build_nc()
        _CACHE["nc"] = nc
    res = run_bass_kernel_spmd(nc, in_maps, core_ids=list(range(N_CORES)))
    return unshard(res.results)

